# revision 38
# baseline (speedup 1.0000x reference)
"""Trainium2 Bass kernel for nn_Block_19095424598462 (dense transformer block
with talking-heads attention).  Data-parallel over batch: 8 cores x B=1.

Algebraic restructuring (host-side, exact):
  Fold LN1 gamma/beta, q/k projections, pre-softmax head mix and 1/sqrt(KD)
  into per-mixed-head G_h [193,193] (193rd = affine dim); fold v projection,
  post-softmax head mix and output projection into V_h [193,192].

All matmuls run in fp8e4 DoubleRow (K=256 per pass, d/s packed in pairs of
partition-planes).  Weights are host-prescaled into fp8 range (G x 4096,
V x 64, W1/W2 x 256); descales ride for free on the ACT scale param or the
final combine multiplies.  exp() of the scores is computed two ways, split
across engines for parallelism:
  - ACT: real exp on [128,1024] PSUM score-pairs -> fp8 (scale=1/S_G free)
  - DVE: Schraudolph-in-fp8-space: bits = round(score * 8/ln2 + 55.5) written
    as int8 == the e4m3 bit pattern of exp(score) (~4% elementwise err, which
    averages out incoherently across the 2048-term softmax sums).
Softmax denominator = extra all-64 column of V (column 192); num and den are
both x64 so the ratio is exact.  Pool (GpSimd) takes memsets and LN scale ops
(it cannot touch PSUM).
"""

import numpy as np
import ml_dtypes

import concourse.bass as bass
import concourse.mybir as mybir
import concourse.tile as tile
from concourse import bacc
from concourse.bass_utils import run_bass_kernel_spmd

F32 = mybir.dt.float32
BF16 = mybir.dt.bfloat16
FP8 = mybir.dt.float8e4
I8 = mybir.dt.int8
PM = mybir.MatmulPerfMode
AF = mybir.ActivationFunctionType
OP = mybir.AluOpType

# --- ACT table-set steering: keep Exp and Ln resolving to the shared set so
# the rstd (Ln+Exp) never forces a table swap inside the exp stream.
_orig_get_tables = None


def _patched_tables(arch):
    tabs = _orig_get_tables(arch)
    keep = "natural_log_exp_and_others"
    if keep in tabs and AF.Exp in tabs[keep] and AF.Ln in tabs[keep]:
        for name, fns in tabs.items():
            if name != keep:
                fns.discard(AF.Exp)
                fns.discard(AF.Ln)
    return tabs


def _install_table_patch():
    global _orig_get_tables
    if _orig_get_tables is None:
        _orig_get_tables = bacc.get_activation_tables
        bacc.get_activation_tables = _patched_tables


P = 128
T = 2048
D = 192
DA = 193          # augmented (affine) contraction dim
TCH = 512         # t-chunk width
NCH = T // TCH    # 4 chunks
TSUB = TCH // P   # 4 subtiles per chunk
NT = T // P       # 16 row tiles
NK = NT // 2      # 8 s-tile pairs
HID = 768
HJ = HID // P     # 6
NA = 3            # fc2 K-blocks of 256
NHEAD = 3
EPS = 1e-3

S_G = 4096.0      # G prescale; bounded by max |S_G * G^T za| < 240 (fp8 max)
S_W = 256.0       # w1/w2 prescale; descaled at gelu / fc2 combine
S_V = 64.0        # V prescale; cancels in num/den ratio (ones col = 64)
SCH_K = 8.0 / np.log(2.0)   # Schraudolph slope (e4m3: 8 bits per octave)
SCH_C = 55.5                # e4m3 exponent bias * 8 - 0.5 (round-to-nearest)

TRACE = False
LAST_RESULTS = None

# exp-pair engine assignment: (h*NK+k) -> True = DVE Schraudolph, False = ACT
def _exp_on_dve(idx):
    return idx % 3 == 1


def _prep_host(inp):
    f8 = np.float64
    wq, wk, wv, wo = (np.asarray(inp[k], f8) for k in ("wq", "wk", "wv", "wo"))
    pre_w, post_w = np.asarray(inp["pre_w"], f8), np.asarray(inp["post_w"], f8)
    g1, b1n = np.asarray(inp["gamma1"], f8), np.asarray(inp["beta1"], f8)
    g2, b2n = np.asarray(inp["gamma2"], f8), np.asarray(inp["beta2"], f8)
    w1, b1 = np.asarray(inp["w1"], f8), np.asarray(inp["b1"], f8)
    w2, b2 = np.asarray(inp["w2"], f8), np.asarray(inp["b2"], f8)
    KD = wq.shape[2]

    G = np.einsum("hg,dhk,ehk->gde", pre_w, wq, wk) / np.sqrt(KD)  # [h,D,D]
    V = np.einsum("hg,dgk,gke->hde", post_w, wv, wo)               # [h,D,D]
    b1p = b1 + b2n @ w1                                            # fold LN2 beta

    # G_aug [h, DA, DA] with affine row/col (supports beta1 != 0)
    G_aug = np.zeros((NHEAD, DA, DA), f8)
    for g in range(NHEAD):
        Gg = G[g]
        G_aug[g, :D, :D] = (g1[:, None] * Gg) * g1[None, :]
        G_aug[g, :D, D] = g1 * (Gg @ b1n)
        G_aug[g, D, :D] = (b1n @ Gg) * g1
        G_aug[g, D, D] = b1n @ Gg @ b1n
    V_aug = np.zeros((NHEAD, DA, D), f8)
    V_aug[:, :D, :] = g1[None, :, None] * V
    V_aug[:, D, :] = b1n @ V
    W1_aug = g2[:, None] * w1                                      # [D, HID]

    bf8 = ml_dtypes.float8_e4m3
    # pair-packed (d = 2p+i) fp8 weights
    # gpk: stationary for nh, additionally pair-packing the OUTPUT dims m in
    # planes par (m = 2q+par), contiguous per par: [P, h, par, i, 97]
    gpk = np.zeros((P, NHEAD, 2, 2, 128), f8)
    vpk = np.zeros((P, NHEAD, 2, D), f8)
    w1pk = np.zeros((P, 2, HID), f8)
    for i in range(2):
        rows = np.arange(i, DA, 2)       # d values for plane i
        for par in range(2):
            cols = np.arange(par, DA, 2)
            blk = (G_aug[:, rows, :][:, :, cols] * S_G).transpose(1, 0, 2)
            gpk[: len(rows), :, par, i, : len(cols)] = blk
        vpk[: len(rows), :, i, :] = (V_aug[:, rows, :] * S_V).transpose(1, 0, 2)
        w1rows = rows[rows < D]
        w1pk[: len(w1rows), i, :] = W1_aug[w1rows, :] * S_W
    # w2 pair-packed over hid: plane i <-> hid = 128*(2a+i)+p
    w2pk = np.zeros((P, NA, 2, D), f8)
    for a in range(NA):
        for i in range(2):
            w2pk[:, a, i, :] = w2[(2 * a + i) * P : (2 * a + i + 1) * P, :] * S_W

    weights = {
        "gpk": gpk.astype(bf8),
        "vpk": vpk.astype(bf8),
        "w1pk": w1pk.astype(bf8),
        "w2pk": w2pk.astype(bf8),
        "b1p": np.ascontiguousarray(
            b1p.reshape(HJ, P).T.astype(np.float32)),     # [P, HJ]
    }
    has_b2 = bool(np.any(b2 != 0.0))
    if has_b2:
        weights["b2bc"] = np.broadcast_to(b2.astype(np.float32), (P, D)).copy()
    return weights, has_b2


def _build(has_b2):
    nc = bacc.Bacc("TRN2", target_bir_lowering=False, debug=False)

    x_d = nc.declare_dram_parameter("x", [T, D], F32, isOutput=False)
    gpk_d = nc.declare_dram_parameter("gpk", [P, NHEAD, 2, 2, 128], FP8, isOutput=False)
    vpk_d = nc.declare_dram_parameter("vpk", [P, NHEAD, 2, D], FP8, isOutput=False)
    w1_d = nc.declare_dram_parameter("w1pk", [P, 2, HID], FP8, isOutput=False)
    w2_d = nc.declare_dram_parameter("w2pk", [P, NA, 2, D], FP8, isOutput=False)
    b1_d = nc.declare_dram_parameter("b1p", [P, HJ], F32, isOutput=False)
    if has_b2:
        b2_d = nc.declare_dram_parameter("b2bc", [P, D], F32, isOutput=False)
    y_d = nc.declare_dram_parameter("y", [T, D], F32, isOutput=True)

    from contextlib import ExitStack
    with tile.TileContext(nc) as tc, ExitStack() as ctx:
        singles = ctx.enter_context(tc.tile_pool(name="singles", bufs=1))
        work = ctx.enter_context(tc.tile_pool(name="work", bufs=4))
        y1p = ctx.enter_context(tc.tile_pool(name="y1p", bufs=1))
        e_pool = ctx.enter_context(tc.tile_pool(name="e_pool", bufs=1))
        nh_pool = ctx.enter_context(tc.tile_pool(name="nh_pool", bufs=1))
        n2_pool = ctx.enter_context(tc.tile_pool(name="n2_pool", bufs=1))
        ht_pool = ctx.enter_context(tc.tile_pool(name="ht_pool", bufs=1))
        ps_w = ctx.enter_context(tc.tile_pool(name="ps_w", bufs=2, space="PSUM"))
        ps_c = ctx.enter_context(tc.tile_pool(name="ps_c", bufs=2, space="PSUM"))
        ps_m = ctx.enter_context(tc.tile_pool(name="ps_m", bufs=2, space="PSUM"))

        # ---- x tiles first (they head the LN1 critical path), then weights
        xa_tiles = {}
        for i in range(NT):
            xa = singles.tile([P, D], F32, name=f"xa{i}")
            nc.sync.dma_start(out=xa, in_=x_d.ap()[i * P:(i + 1) * P, :])
            xa_tiles[i] = xa

        # ---- constants into SBUF
        gpk = singles.tile([P, NHEAD, 2, 2, 128], FP8)
        nc.sync.dma_start(out=gpk, in_=gpk_d.ap())
        vpk = singles.tile([P, NHEAD, 2, D], FP8)
        nc.sync.dma_start(out=vpk, in_=vpk_d.ap())
        w1pk = singles.tile([P, 2, HID], FP8)
        nc.sync.dma_start(out=w1pk, in_=w1_d.ap())
        w2pk = singles.tile([P, NA, 2, D], FP8)
        nc.sync.dma_start(out=w2pk, in_=w2_d.ap())
        b1sb = singles.tile([P, HJ], F32)
        nc.sync.dma_start(out=b1sb, in_=b1_d.ap())
        # identity for PE transposes, built on-device (a [128,128] DMA of
        # 256-byte lines costs ~10us on the slow direct-2D path)
        ident = singles.tile([P, P], BF16)
        ii32 = singles.tile([P, P], mybir.dt.int32)
        nc.gpsimd.iota(ii32, pattern=[[-1, P]], base=0, channel_multiplier=1)
        nc.gpsimd.tensor_scalar(out=ident, in0=ii32, scalar1=0, scalar2=None,
                                op0=OP.is_equal)
        if has_b2:
            b2sb = singles.tile([P, D], F32)
            nc.sync.dma_start(out=b2sb, in_=b2_d.ap())
        eps_sb = singles.tile([P, 1], F32)
        nc.gpsimd.memset(eps_sb, EPS)
        tw = work.tile([P, 1], F32, tag="tw")
        nc.scalar.activation(out=tw, in_=eps_sb, func=AF.Ln, bias=eps_sb)

        # zT fp8 pair-packed: zpk[p, i, t] = z_aug[t, 2p+i]; affine row: d=192
        # = (96, plane 0) = 1.0; rows 96..127 otherwise zero.
        zpk = singles.tile([P, 2, T], FP8)
        nc.gpsimd.memset(zpk[96:128, :, :], 0.0)
        nc.gpsimd.memset(zpk[96:97, 0, :], 1.0)

        # v-tilde pair-packed over s: vtp[p, h, k, i, :] = row s=(2k+i)*128+p;
        # column 192 = S_V (denominator column; num/den both x S_V).
        vtp = singles.tile([P, NHEAD, NK, 2, DA], FP8)
        nc.gpsimd.memset(vtp[:, :, :, :, D:DA], S_V)

        # nh fp8 pair-packed over mixed dims m: nhpk[p, par, g, t] = nh[2p+par, t]
        nhpk = nh_pool.tile([P, 2, NHEAD, TCH], FP8)
        nc.gpsimd.memset(nhpk[96:128, :, :, :], 0.0)

        def ln_stats(src_ap, mv_slice):
            st = work.tile([P, 6], F32, tag="bnst")
            nc.vector.bn_stats(out=st, in_=src_ap)
            nc.vector.bn_aggr(out=mv_slice, in_=st)

        def ln_rstd_batch(mv_all, rstd_all, n):
            lnv = work.tile([P, n], F32, tag=f"lnv{n}")
            nc.scalar.activation(out=lnv, in_=mv_all[:, :n, 1], func=AF.Ln,
                                 bias=eps_sb)
            nc.scalar.activation(out=rstd_all[:, :n], in_=lnv, func=AF.Exp,
                                 scale=-0.5)

        def ln_z(src_ap, mv_slice, rstd_ap, tag):
            # (x - mu) * rstd, bf16 out for PE transpose
            z = work.tile([P, D], BF16, tag=tag)
            nc.vector.tensor_scalar(
                out=z, in0=src_ap, scalar1=mv_slice[:, 0:1], scalar2=rstd_ap,
                op0=OP.subtract, op1=OP.mult,
            )
            return z

        def ln_z_act(src_ap, rstd_ap, negmupr_ap, tag):
            # (x - mu) * rstd = x*rstd + (-mu*rstd), on ACT (Identity is in
            # every table set -> no table swap); bf16 out for PE transpose
            z = work.tile([P, D], BF16, tag=tag)
            nc.scalar.activation(out=z, in_=src_ap, func=AF.Identity,
                                 scale=rstd_ap, bias=negmupr_ap)
            return z

        def neg_mu_rstd(mv_all, rstd_all, n, tag):
            # -mu * rstd for a batch of n tiles: one DVE op [P, n]
            nm = work.tile([P, n], F32, tag=tag)
            nc.vector.scalar_tensor_tensor(
                out=nm, in0=mv_all[:, :n, 0], scalar=-1.0, in1=rstd_all[:, :n],
                op0=OP.mult, op1=OP.mult)
            return nm

        copy_flip = [0]

        def copy_alt(out_ap, in_ap):
            # alternate PSUM->SBUF drains between DVE and ACT
            copy_flip[0] ^= 1
            if copy_flip[0]:
                nc.vector.tensor_copy(out=out_ap, in_=in_ap)
            else:
                nc.scalar.copy(out=out_ap, in_=in_ap)

        def transpose_pack(z, dst, col):
            """z [128, D(bf16)] -> dst planes: dst[0:96, i, col:col+128];
            both plane transposes land in one PSUM tile, drained by one copy."""
            pt = ps_m.tile([P, TCH], BF16, tag="ps_m", name="ps_m")
            nc.tensor.transpose(pt[:96, 0:P], z[:, 0:D:2], ident)
            nc.tensor.transpose(pt[:96, P:2 * P], z[:, 1:D:2], ident)
            src = pt[:96, 0:2 * P].rearrange("p (i t) -> p i t", i=2)
            copy_alt(dst[0:96, :, col:col + P], src)

        # ---- Phase A: LN1 -> zpk, interleaved with Phase B v-tilde.
        # rstd in batches of 4 so transposes start after 4 tiles, not 16.
        mv1 = singles.tile([P, NT, 2], F32)
        rstd1 = singles.tile([P, NT], F32)
        def emit_phase_a_batch(b0, bn_):
            for i in range(b0, b0 + bn_):
                ln_stats(xa_tiles[i], mv1[:, i, :])
            ln_rstd_batch(mv1[:, b0:b0 + bn_, :], rstd1[:, b0:b0 + bn_], bn_)
            nm1 = neg_mu_rstd(mv1[:, b0:b0 + bn_, :], rstd1[:, b0:b0 + bn_],
                              bn_, f"nm1_{b0}")
            for i in range(b0, b0 + bn_):
                if i % 2 == 0:  # even tiles on ACT (idle in phase A), odd DVE
                    z = ln_z_act(xa_tiles[i], rstd1[:, i:i + 1],
                                 nm1[:, i - b0:i - b0 + 1], f"z1_{i % 4}")
                else:
                    z = ln_z(xa_tiles[i], mv1[:, i, :], rstd1[:, i:i + 1],
                             f"z1_{i % 4}")
                transpose_pack(z, zpk, i * P)
                # v-tilde for s-tile i, all 3 heads into one wide PSUM tile
                # (bank-aligned 256-col windows), drained by one strided copy
                pv = ps_w.tile([P, 2 * TCH], F32, tag="ps_w", name="ps_w")
                for h in range(NHEAD):
                    nc.tensor.matmul(pv[:, h * 256:h * 256 + D],
                                     lhsT=zpk[:, :, i * P:(i + 1) * P],
                                     rhs=vpk[:, h, :, :], start=True, stop=True,
                                     perf_mode=PM.DoubleRow)
                sap = pv[:, 0:768].rearrange("p (h q) -> p h q", h=3)[:, :, 0:D]
                copy_alt(vtp[:, :, i // 2, i % 2, 0:D], sap)

        # ---- chunk loop
        y1_tiles = {}
        n2pk_tiles = {}

        def emit_fc1(cc):
            n2pk = n2pk_tiles[cc]
            htpk = []
            for a in range(NA):
                ht = ht_pool.tile([P, 2, TCH], FP8, tag=f"ht{a}", name=f"ht{a}")
                htpk.append(ht)
            for j in range(HJ):
                pm = ps_m.tile([P, TCH], F32, tag="ps_m", name="ps_m")
                nc.tensor.matmul(pm, lhsT=w1pk[:, :, j * P:(j + 1) * P],
                                 rhs=n2pk, start=True, stop=True,
                                 perf_mode=PM.DoubleRow)
                nc.scalar.activation(out=htpk[j // 2][:, j % 2, :], in_=pm,
                                     func=AF.Gelu, bias=b1sb[:, j:j + 1],
                                     scale=1.0 / S_W)
            return htpk

        def emit_fc2(cc, htpk):
            for ts in range(TSUB):
                ti = cc * TSUB + ts
                pf = ps_m.tile([P, TCH], F32, tag="ps_m", name="ps_m")
                for a in range(NA):
                    nc.tensor.matmul(pf[:, 0:D],
                                     lhsT=htpk[a][:, :, ts * P:(ts + 1) * P],
                                     rhs=w2pk[:, a, :, :],
                                     start=(a == 0), stop=(a == NA - 1),
                                     perf_mode=PM.DoubleRow)
                ot = work.tile([P, D], F32, tag=f"out{ts}")
                nc.vector.scalar_tensor_tensor(
                    out=ot, in0=pf[:, 0:D], scalar=1.0 / S_W,
                    in1=y1_tiles[ti], op0=OP.mult, op1=OP.add)
                if has_b2:
                    nc.vector.tensor_tensor(out=ot, in0=ot, in1=b2sb, op=OP.add)
                nc.sync.dma_start(out=y_d.ap()[ti * P:(ti + 1) * P, :], in_=ot)

        e_tiles = {}

        def emit_scores_head(c, g, ks=None):
            for k in (range(NK) if ks is None else ks):
                pw = ps_w.tile([P, 2 * TCH], F32, tag="ps_w", name="ps_w")
                for i in range(2):
                    s = 2 * k + i
                    nc.tensor.matmul(pw[:, i * TCH:(i + 1) * TCH],
                                     lhsT=zpk[:, :, s * P:(s + 1) * P],
                                     rhs=nhpk[:, :, g, :],
                                     start=True, stop=True,
                                     perf_mode=PM.DoubleRow)
                et = e_pool.tile([P, 2, TCH], FP8, tag=f"e{g}_{k}",
                                 name=f"e{g}_{k}")
                et_flat = et.rearrange("p i t -> p (i t)")
                if _exp_on_dve(g * NK + k):
                    nc.vector.tensor_scalar(
                        out=et_flat.bitcast(I8), in0=pw,
                        scalar1=SCH_K / S_G, scalar2=SCH_C,
                        op0=OP.mult, op1=OP.add)
                else:
                    nc.scalar.activation(out=et_flat, in_=pw, func=AF.Exp,
                                         scale=1.0 / S_G)
                e_tiles[(g, k)] = et

        def emit_ctx_head(c, g, mv2=None):
            for ts in range(TSUB):
                ti = c * TSUB + ts
                pc = ps_c.tile([P, TCH], F32, tag="ps_c", name="ps_c")
                for k in range(NK):
                    nc.tensor.matmul(
                        pc[:, 0:DA],
                        lhsT=e_tiles[(g, k)][:, :, ts * P:(ts + 1) * P],
                        rhs=vtp[:, g, k, :, :],
                        start=(k == 0), stop=(k == NK - 1),
                        perf_mode=PM.DoubleRow)
                rc = work.tile([P, 1], F32, tag=f"rc{ts}")
                nc.vector.reciprocal(out=rc, in_=pc[:, D:DA])
                nc.vector.scalar_tensor_tensor(
                    out=y1_tiles[ti], in0=pc[:, 0:D], scalar=rc,
                    in1=(xa_tiles[ti] if g == 0 else y1_tiles[ti]),
                    op0=OP.mult, op1=OP.add)
                if mv2 is not None:
                    ln_stats(y1_tiles[ti], mv2[:, ts, :])

        def emit_nh(c, heads=(0, 1, 2), drain="dve"):
            csl = slice(c * TCH, (c + 1) * TCH)
            for g in heads:
                for par, mw in ((0, 97), (1, 96)):
                    pn = ps_m.tile([P, TCH], F32, tag="ps_m", name="ps_m")
                    nc.tensor.matmul(pn[:mw, :], lhsT=gpk[:, g, par, :, :mw],
                                     rhs=zpk[:, :, csl], start=True, stop=True,
                                     perf_mode=PM.DoubleRow)
                    if drain == "dve":
                        nc.vector.tensor_copy(out=nhpk[0:mw, par, g, :],
                                              in_=pn[:mw, :])
                    else:
                        nc.scalar.copy(out=nhpk[0:mw, par, g, :], in_=pn[:mw, :])

        def alloc_y1(c):
            for ts in range(TSUB):
                ti = c * TSUB + ts
                y1_tiles[ti] = y1p.tile([P, D], F32, name=f"y1_{ti}")

        def emit_ln2(c, mv2, rstd2):
            ln_rstd_batch(mv2, rstd2, TSUB)
            if c == NCH - 1:
                # last chunk: prewarm the gelu table while DVE runs ln_z (for
                # other chunks the load hides under the next exp stream)
                nc.scalar.activation(out=tw, in_=eps_sb, func=AF.Gelu)
            n2pk = n2_pool.tile([P, 2, TCH], FP8, tag="n2pk", name="n2pk")
            if c == 0:
                # pad rows (d >= 192): zero once; fp8 garbage here would
                # poison fc1 psum via NaN * 0
                nc.gpsimd.memset(n2pk[96:128, :, :], 0.0)
            for ts in range(TSUB):
                ti = c * TSUB + ts
                z2 = ln_z(y1_tiles[ti], mv2[:, ts, :], rstd2[:, ts:ts + 1],
                          f"z2_{ts}")
                transpose_pack(z2, n2pk, ts * P)
            n2pk_tiles[c] = n2pk

        # --- two-deep software pipeline: scores/exps of chunk c+1 are issued
        # during chunk c's ctx/LN2/MLP windows so ACT/DVE never idle there.
        emit_phase_a_batch(0, 2)
        emit_phase_a_batch(2, 2)
        emit_nh(0)
        alloc_y1(0)
        emit_phase_a_batch(4, 4)
        emit_scores_head(0, 0, ks=[0, 1, 2, 3])
        emit_phase_a_batch(8, 4)
        emit_scores_head(0, 0, ks=[4, 5])
        emit_scores_head(0, 1, ks=[0, 1, 2, 3])
        emit_phase_a_batch(12, 4)
        emit_scores_head(0, 0, ks=[6, 7])
        emit_scores_head(0, 1, ks=[4, 5, 6, 7])
        emit_ctx_head(0, 0)
        emit_scores_head(0, 2)
        emit_nh(1)
        emit_ctx_head(0, 1)
        for c in range(NCH):
            mv2 = work.tile([P, TSUB, 2], F32, tag="mv2")
            rstd2 = work.tile([P, TSUB], F32, tag="rstd2")
            if c + 1 < NCH:
                alloc_y1(c + 1)
                emit_scores_head(c + 1, 0)
            emit_ctx_head(c, 2, mv2=mv2)
            emit_ln2(c, mv2, rstd2)
            if c + 1 < NCH:
                # split score-head blocks around independent PE work: the PE
                # outruns the exp drains ~2 pairs into a block (ps_w bufs=2),
                # so fc1/fc2/nh land exactly where it would stall
                emit_scores_head(c + 1, 1, ks=[0, 1, 2, 3])
                ht = emit_fc1(c)
                emit_scores_head(c + 1, 1, ks=[4, 5, 6, 7])
                emit_fc2(c, ht)
                emit_ctx_head(c + 1, 0)
                emit_scores_head(c + 1, 2, ks=[0, 1, 2, 3])
                if c + 2 < NCH:
                    emit_nh(c + 2)
                emit_scores_head(c + 1, 2, ks=[4, 5, 6, 7])
                emit_ctx_head(c + 1, 1)
            else:
                emit_fc2(c, emit_fc1(c))

    nc.finalize()
    return nc


_module_cache = {}


def kernel(**inputs):
    global LAST_RESULTS
    x = np.ascontiguousarray(np.asarray(inputs["x"], np.float32))
    B = x.shape[0]
    assert x.shape == (B, T, D) and B == 8

    weights, has_b2 = _prep_host(inputs)

    _install_table_patch()
    if has_b2 not in _module_cache:
        _module_cache[has_b2] = _build(has_b2)
    nc = _module_cache[has_b2]

    in_maps = [dict(weights, x=x[b]) for b in range(B)]
    res = run_bass_kernel_spmd(nc, in_maps, core_ids=list(range(B)), trace=TRACE)
    LAST_RESULTS = res
    out = np.stack([np.asarray(res.results[b]["y"], np.float32) for b in range(B)])
    return out


# revision 39
# speedup vs baseline: 1.0369x; 1.0369x over previous
"""Trainium2 Bass kernel for nn_Block_19095424598462 (dense transformer block
with talking-heads attention).  Data-parallel over batch: 8 cores x B=1.

Algebraic restructuring (host-side, exact):
  Fold LN1 gamma/beta, q/k projections, pre-softmax head mix and 1/sqrt(KD)
  into per-mixed-head G_h [193,193] (193rd = affine dim); fold v projection,
  post-softmax head mix and output projection into V_h [193,192].

All matmuls run in fp8e4 DoubleRow (K=256 per pass, d/s packed in pairs of
partition-planes).  Weights are host-prescaled into fp8 range (G x 4096,
V x 64, W1/W2 x 256); descales ride for free on the ACT scale param or the
final combine multiplies.  exp() of the scores is computed two ways, split
across engines for parallelism:
  - ACT: real exp on [128,1024] PSUM score-pairs -> fp8 (scale=1/S_G free)
  - DVE: Schraudolph-in-fp8-space: bits = round(score * 8/ln2 + 55.5) written
    as int8 == the e4m3 bit pattern of exp(score) (~4% elementwise err, which
    averages out incoherently across the 2048-term softmax sums).
Softmax denominator = extra all-64 column of V (column 192); num and den are
both x64 so the ratio is exact.  Pool (GpSimd) takes memsets and LN scale ops
(it cannot touch PSUM).
"""

import numpy as np
import ml_dtypes

import concourse.bass as bass
import concourse.mybir as mybir
import concourse.tile as tile
from concourse import bacc
from concourse.bass_utils import run_bass_kernel_spmd

F32 = mybir.dt.float32
BF16 = mybir.dt.bfloat16
FP8 = mybir.dt.float8e4
I8 = mybir.dt.int8
PM = mybir.MatmulPerfMode
AF = mybir.ActivationFunctionType
OP = mybir.AluOpType

# --- ACT table-set steering: keep Exp and Ln resolving to the shared set so
# the rstd (Ln+Exp) never forces a table swap inside the exp stream.
_orig_get_tables = None


def _patched_tables(arch):
    tabs = _orig_get_tables(arch)
    keep = "natural_log_exp_and_others"
    if keep in tabs and AF.Exp in tabs[keep] and AF.Ln in tabs[keep]:
        for name, fns in tabs.items():
            if name != keep:
                fns.discard(AF.Exp)
                fns.discard(AF.Ln)
    return tabs


def _install_table_patch():
    global _orig_get_tables
    if _orig_get_tables is None:
        _orig_get_tables = bacc.get_activation_tables
        bacc.get_activation_tables = _patched_tables


P = 128
T = 2048
D = 192
DA = 193          # augmented (affine) contraction dim
TCH = 512         # t-chunk width
NCH = T // TCH    # 4 chunks
TSUB = TCH // P   # 4 subtiles per chunk
NT = T // P       # 16 row tiles
NK = NT // 2      # 8 s-tile pairs
HID = 768
HJ = HID // P     # 6
NA = 3            # fc2 K-blocks of 256
NHEAD = 3
EPS = 1e-3

S_G = 4096.0      # G prescale; bounded by max |S_G * G^T za| < 240 (fp8 max)
S_W = 256.0       # w1/w2 prescale; descaled at gelu / fc2 combine
S_V = 64.0        # V prescale; cancels in num/den ratio (ones col = 64)
SCH_K = 8.0 / np.log(2.0)   # Schraudolph slope (e4m3: 8 bits per octave)
SCH_C = 55.5                # e4m3 exponent bias * 8 - 0.5 (round-to-nearest)

TRACE = False
LAST_RESULTS = None

# exp-pair engine assignment: (h*NK+k) -> True = DVE Schraudolph, False = ACT
def _exp_on_dve(idx):
    return idx % 3 == 1


def _prep_host(inp):
    f8 = np.float64
    wq, wk, wv, wo = (np.asarray(inp[k], f8) for k in ("wq", "wk", "wv", "wo"))
    pre_w, post_w = np.asarray(inp["pre_w"], f8), np.asarray(inp["post_w"], f8)
    g1, b1n = np.asarray(inp["gamma1"], f8), np.asarray(inp["beta1"], f8)
    g2, b2n = np.asarray(inp["gamma2"], f8), np.asarray(inp["beta2"], f8)
    w1, b1 = np.asarray(inp["w1"], f8), np.asarray(inp["b1"], f8)
    w2, b2 = np.asarray(inp["w2"], f8), np.asarray(inp["b2"], f8)
    KD = wq.shape[2]

    G = np.einsum("hg,dhk,ehk->gde", pre_w, wq, wk) / np.sqrt(KD)  # [h,D,D]
    V = np.einsum("hg,dgk,gke->hde", post_w, wv, wo)               # [h,D,D]
    b1p = b1 + b2n @ w1                                            # fold LN2 beta

    # G_aug [h, DA, DA] with affine row/col (supports beta1 != 0)
    G_aug = np.zeros((NHEAD, DA, DA), f8)
    for g in range(NHEAD):
        Gg = G[g]
        G_aug[g, :D, :D] = (g1[:, None] * Gg) * g1[None, :]
        G_aug[g, :D, D] = g1 * (Gg @ b1n)
        G_aug[g, D, :D] = (b1n @ Gg) * g1
        G_aug[g, D, D] = b1n @ Gg @ b1n
    V_aug = np.zeros((NHEAD, DA, D), f8)
    V_aug[:, :D, :] = g1[None, :, None] * V
    V_aug[:, D, :] = b1n @ V
    W1_aug = g2[:, None] * w1                                      # [D, HID]

    bf8 = ml_dtypes.float8_e4m3
    # pair-packed (d = 2p+i) fp8 weights
    # gpk: stationary for nh, additionally pair-packing the OUTPUT dims m in
    # planes par (m = 2q+par), contiguous per par: [P, h, par, i, 97]
    gpk = np.zeros((P, NHEAD, 2, 2, 128), f8)
    vpk = np.zeros((P, NHEAD, 2, D), f8)
    w1pk = np.zeros((P, 2, HID), f8)
    for i in range(2):
        rows = np.arange(i, DA, 2)       # d values for plane i
        for par in range(2):
            cols = np.arange(par, DA, 2)
            blk = (G_aug[:, rows, :][:, :, cols] * S_G).transpose(1, 0, 2)
            gpk[: len(rows), :, par, i, : len(cols)] = blk
        vpk[: len(rows), :, i, :] = (V_aug[:, rows, :] * S_V).transpose(1, 0, 2)
        w1rows = rows[rows < D]
        w1pk[: len(w1rows), i, :] = W1_aug[w1rows, :] * S_W
    # w2 pair-packed over hid: plane i <-> hid = 128*(2a+i)+p
    w2pk = np.zeros((P, NA, 2, D), f8)
    for a in range(NA):
        for i in range(2):
            w2pk[:, a, i, :] = w2[(2 * a + i) * P : (2 * a + i + 1) * P, :] * S_W

    weights = {
        "gpk": gpk.astype(bf8),
        "vpk": vpk.astype(bf8),
        "w1pk": w1pk.astype(bf8),
        "w2pk": w2pk.astype(bf8),
        "b1p": np.ascontiguousarray(
            b1p.reshape(HJ, P).T.astype(np.float32)),     # [P, HJ]
    }
    has_b2 = bool(np.any(b2 != 0.0))
    if has_b2:
        weights["b2bc"] = np.broadcast_to(b2.astype(np.float32), (P, D)).copy()
    return weights, has_b2


def _build(has_b2):
    nc = bacc.Bacc("TRN2", target_bir_lowering=False, debug=False)

    x_d = nc.declare_dram_parameter("x", [T, D], F32, isOutput=False)
    gpk_d = nc.declare_dram_parameter("gpk", [P, NHEAD, 2, 2, 128], FP8, isOutput=False)
    vpk_d = nc.declare_dram_parameter("vpk", [P, NHEAD, 2, D], FP8, isOutput=False)
    w1_d = nc.declare_dram_parameter("w1pk", [P, 2, HID], FP8, isOutput=False)
    w2_d = nc.declare_dram_parameter("w2pk", [P, NA, 2, D], FP8, isOutput=False)
    b1_d = nc.declare_dram_parameter("b1p", [P, HJ], F32, isOutput=False)
    if has_b2:
        b2_d = nc.declare_dram_parameter("b2bc", [P, D], F32, isOutput=False)
    y_d = nc.declare_dram_parameter("y", [T, D], F32, isOutput=True)

    from contextlib import ExitStack
    with tile.TileContext(nc) as tc, ExitStack() as ctx:
        singles = ctx.enter_context(tc.tile_pool(name="singles", bufs=1))
        work = ctx.enter_context(tc.tile_pool(name="work", bufs=4))
        y1p = ctx.enter_context(tc.tile_pool(name="y1p", bufs=1))
        e_pool = ctx.enter_context(tc.tile_pool(name="e_pool", bufs=1))
        nh_pool = ctx.enter_context(tc.tile_pool(name="nh_pool", bufs=1))
        n2_pool = ctx.enter_context(tc.tile_pool(name="n2_pool", bufs=1))
        ht_pool = ctx.enter_context(tc.tile_pool(name="ht_pool", bufs=1))
        ps_w = ctx.enter_context(tc.tile_pool(name="ps_w", bufs=2, space="PSUM"))
        ps_c = ctx.enter_context(tc.tile_pool(name="ps_c", bufs=2, space="PSUM"))
        ps_m = ctx.enter_context(tc.tile_pool(name="ps_m", bufs=2, space="PSUM"))

        # ---- x tiles first (they head the LN1 critical path), then weights
        xa_tiles = {}
        for i in range(NT):
            xa = singles.tile([P, D], F32, name=f"xa{i}")
            nc.sync.dma_start(out=xa, in_=x_d.ap()[i * P:(i + 1) * P, :])
            xa_tiles[i] = xa

        # ---- constants into SBUF
        gpk = singles.tile([P, NHEAD, 2, 2, 128], FP8)
        nc.sync.dma_start(out=gpk, in_=gpk_d.ap())
        vpk = singles.tile([P, NHEAD, 2, D], FP8)
        nc.sync.dma_start(out=vpk, in_=vpk_d.ap())
        w1pk = singles.tile([P, 2, HID], FP8)
        nc.sync.dma_start(out=w1pk, in_=w1_d.ap())
        w2pk = singles.tile([P, NA, 2, D], FP8)
        nc.sync.dma_start(out=w2pk, in_=w2_d.ap())
        b1sb = singles.tile([P, HJ], F32)
        nc.sync.dma_start(out=b1sb, in_=b1_d.ap())
        # identity for PE transposes, built on-device (a [128,128] DMA of
        # 256-byte lines costs ~10us on the slow direct-2D path)
        ident = singles.tile([P, P], BF16)
        ii32 = singles.tile([P, P], mybir.dt.int32)
        nc.gpsimd.iota(ii32, pattern=[[-1, P]], base=0, channel_multiplier=1)
        nc.gpsimd.tensor_scalar(out=ident, in0=ii32, scalar1=0, scalar2=None,
                                op0=OP.is_equal)
        if has_b2:
            b2sb = singles.tile([P, D], F32)
            nc.sync.dma_start(out=b2sb, in_=b2_d.ap())
        eps_sb = singles.tile([P, 1], F32)
        nc.gpsimd.memset(eps_sb, EPS)
        tw = work.tile([P, 1], F32, tag="tw")
        nc.scalar.activation(out=tw, in_=eps_sb, func=AF.Ln, bias=eps_sb)

        # zT fp8 pair-packed: zpk[p, i, t] = z_aug[t, 2p+i]; affine row: d=192
        # = (96, plane 0) = 1.0; rows 96..127 otherwise zero.
        zpk = singles.tile([P, 2, T], FP8)
        nc.gpsimd.memset(zpk[96:128, :, :], 0.0)
        nc.gpsimd.memset(zpk[96:97, 0, :], 1.0)

        # v-tilde pair-packed over s: vtp[p, h, k, i, :] = row s=(2k+i)*128+p;
        # column 192 = S_V (denominator column; num/den both x S_V).
        vtp = singles.tile([P, NHEAD, NK, 2, DA], FP8)
        nc.gpsimd.memset(vtp[:, :, :, :, D:DA], S_V)

        # nh fp8 pair-packed over mixed dims m: nhpk[p, par, g, t] = nh[2p+par, t]
        nhpk = nh_pool.tile([P, 2, NHEAD, TCH], FP8)
        nc.gpsimd.memset(nhpk[96:128, :, :, :], 0.0)

        def ln_stats(src_ap, mv_slice):
            st = work.tile([P, 6], F32, tag="bnst")
            nc.vector.bn_stats(out=st, in_=src_ap)
            nc.vector.bn_aggr(out=mv_slice, in_=st)

        def ln_rstd_batch(mv_all, rstd_all, n):
            lnv = work.tile([P, n], F32, tag=f"lnv{n}")
            nc.scalar.activation(out=lnv, in_=mv_all[:, :n, 1], func=AF.Ln,
                                 bias=eps_sb)
            nc.scalar.activation(out=rstd_all[:, :n], in_=lnv, func=AF.Exp,
                                 scale=-0.5)

        def ln_z(src_ap, mv_slice, rstd_ap, tag):
            # (x - mu) * rstd, bf16 out for PE transpose
            z = work.tile([P, D], BF16, tag=tag)
            nc.vector.tensor_scalar(
                out=z, in0=src_ap, scalar1=mv_slice[:, 0:1], scalar2=rstd_ap,
                op0=OP.subtract, op1=OP.mult,
            )
            return z

        def ln_z_act(src_ap, rstd_ap, negmupr_ap, tag):
            # (x - mu) * rstd = x*rstd + (-mu*rstd), on ACT (Identity is in
            # every table set -> no table swap); bf16 out for PE transpose
            z = work.tile([P, D], BF16, tag=tag)
            nc.scalar.activation(out=z, in_=src_ap, func=AF.Identity,
                                 scale=rstd_ap, bias=negmupr_ap)
            return z

        def neg_mu_rstd(mv_all, rstd_all, n, tag):
            # -mu * rstd for a batch of n tiles: one DVE op [P, n]
            nm = work.tile([P, n], F32, tag=tag)
            nc.vector.scalar_tensor_tensor(
                out=nm, in0=mv_all[:, :n, 0], scalar=-1.0, in1=rstd_all[:, :n],
                op0=OP.mult, op1=OP.mult)
            return nm

        copy_flip = [0]

        def copy_alt(out_ap, in_ap):
            # alternate PSUM->SBUF drains between DVE and ACT
            copy_flip[0] ^= 1
            if copy_flip[0]:
                nc.vector.tensor_copy(out=out_ap, in_=in_ap)
            else:
                nc.scalar.copy(out=out_ap, in_=in_ap)

        def transpose_pack(z, dst, col):
            """z [128, D(bf16)] -> dst planes: dst[0:96, i, col:col+128];
            both plane transposes land in one PSUM tile, drained by one copy."""
            pt = ps_m.tile([P, TCH], BF16, tag="ps_m", name="ps_m")
            nc.tensor.transpose(pt[:96, 0:P], z[:, 0:D:2], ident)
            nc.tensor.transpose(pt[:96, P:2 * P], z[:, 1:D:2], ident)
            src = pt[:96, 0:2 * P].rearrange("p (i t) -> p i t", i=2)
            copy_alt(dst[0:96, :, col:col + P], src)

        # ---- Phase A: LN1 -> zpk, interleaved with Phase B v-tilde.
        # rstd in batches of 4 so transposes start after 4 tiles, not 16.
        mv1 = singles.tile([P, NT, 2], F32)
        rstd1 = singles.tile([P, NT], F32)
        def emit_phase_a_batch(b0, bn_):
            for i in range(b0, b0 + bn_):
                ln_stats(xa_tiles[i], mv1[:, i, :])
            ln_rstd_batch(mv1[:, b0:b0 + bn_, :], rstd1[:, b0:b0 + bn_], bn_)
            nm1 = neg_mu_rstd(mv1[:, b0:b0 + bn_, :], rstd1[:, b0:b0 + bn_],
                              bn_, f"nm1_{b0}")
            for i in range(b0, b0 + bn_):
                if i % 2 == 0:  # even tiles on ACT (idle in phase A), odd DVE
                    z = ln_z_act(xa_tiles[i], rstd1[:, i:i + 1],
                                 nm1[:, i - b0:i - b0 + 1], f"z1_{i % 4}")
                else:
                    z = ln_z(xa_tiles[i], mv1[:, i, :], rstd1[:, i:i + 1],
                             f"z1_{i % 4}")
                transpose_pack(z, zpk, i * P)
                # v-tilde for s-tile i, all 3 heads into one wide PSUM tile
                # (bank-aligned 256-col windows), drained by one strided copy
                pv = ps_w.tile([P, 2 * TCH], F32, tag="ps_w", name="ps_w")
                for h in range(NHEAD):
                    nc.tensor.matmul(pv[:, h * 256:h * 256 + D],
                                     lhsT=zpk[:, :, i * P:(i + 1) * P],
                                     rhs=vpk[:, h, :, :], start=True, stop=True,
                                     perf_mode=PM.DoubleRow)
                sap = pv[:, 0:768].rearrange("p (h q) -> p h q", h=3)[:, :, 0:D]
                copy_alt(vtp[:, :, i // 2, i % 2, 0:D], sap)

        # ---- chunk loop
        y1_tiles = {}
        n2pk_tiles = {}

        def emit_fc1(cc):
            n2pk = n2pk_tiles[cc]
            htpk = []
            for a in range(NA):
                ht = ht_pool.tile([P, 2, TCH], FP8, tag=f"ht{a}", name=f"ht{a}")
                htpk.append(ht)
            for j in range(HJ):
                pm = ps_m.tile([P, TCH], F32, tag="ps_m", name="ps_m")
                nc.tensor.matmul(pm, lhsT=w1pk[:, :, j * P:(j + 1) * P],
                                 rhs=n2pk, start=True, stop=True,
                                 perf_mode=PM.DoubleRow)
                nc.scalar.activation(out=htpk[j // 2][:, j % 2, :], in_=pm,
                                     func=AF.Gelu, bias=b1sb[:, j:j + 1],
                                     scale=1.0 / S_W)
            return htpk

        def emit_fc2(cc, htpk):
            for ts in range(TSUB):
                ti = cc * TSUB + ts
                pf = ps_m.tile([P, TCH], F32, tag="ps_m", name="ps_m")
                for a in range(NA):
                    nc.tensor.matmul(pf[:, 0:D],
                                     lhsT=htpk[a][:, :, ts * P:(ts + 1) * P],
                                     rhs=w2pk[:, a, :, :],
                                     start=(a == 0), stop=(a == NA - 1),
                                     perf_mode=PM.DoubleRow)
                ot = work.tile([P, D], F32, tag=f"out{ts}")
                nc.vector.scalar_tensor_tensor(
                    out=ot, in0=pf[:, 0:D], scalar=1.0 / S_W,
                    in1=y1_tiles[ti], op0=OP.mult, op1=OP.add)
                if has_b2:
                    nc.vector.tensor_tensor(out=ot, in0=ot, in1=b2sb, op=OP.add)
                nc.sync.dma_start(out=y_d.ap()[ti * P:(ti + 1) * P, :], in_=ot)

        e_tiles = {}

        def emit_scores_head(c, g, ks=None):
            for k in (range(NK) if ks is None else ks):
                pw = ps_w.tile([P, 2 * TCH], F32, tag="ps_w", name="ps_w")
                for i in range(2):
                    s = 2 * k + i
                    nc.tensor.matmul(pw[:, i * TCH:(i + 1) * TCH],
                                     lhsT=zpk[:, :, s * P:(s + 1) * P],
                                     rhs=nhpk[:, :, g, :],
                                     start=True, stop=True,
                                     perf_mode=PM.DoubleRow)
                et = e_pool.tile([P, 2, TCH], FP8, tag=f"e{g}_{k}",
                                 name=f"e{g}_{k}")
                et_flat = et.rearrange("p i t -> p (i t)")
                if _exp_on_dve(g * NK + k):
                    nc.vector.tensor_scalar(
                        out=et_flat.bitcast(I8), in0=pw,
                        scalar1=SCH_K / S_G, scalar2=SCH_C,
                        op0=OP.mult, op1=OP.add)
                else:
                    nc.scalar.activation(out=et_flat, in_=pw, func=AF.Exp,
                                         scale=1.0 / S_G)
                e_tiles[(g, k)] = et

        def emit_ctx_head(c, g, mv2=None):
            for ts in range(TSUB):
                ti = c * TSUB + ts
                pc = ps_c.tile([P, TCH], F32, tag="ps_c", name="ps_c")
                for k in range(NK):
                    nc.tensor.matmul(
                        pc[:, 0:DA],
                        lhsT=e_tiles[(g, k)][:, :, ts * P:(ts + 1) * P],
                        rhs=vtp[:, g, k, :, :],
                        start=(k == 0), stop=(k == NK - 1),
                        perf_mode=PM.DoubleRow)
                rc = work.tile([P, 1], F32, tag=f"rc{ts}")
                nc.vector.reciprocal(out=rc, in_=pc[:, D:DA])
                nc.vector.scalar_tensor_tensor(
                    out=y1_tiles[ti], in0=pc[:, 0:D], scalar=rc,
                    in1=(xa_tiles[ti] if g == 0 else y1_tiles[ti]),
                    op0=OP.mult, op1=OP.add)
                if mv2 is not None:
                    ln_stats(y1_tiles[ti], mv2[:, ts, :])

        def emit_nh(c, heads=(0, 1, 2), drain="dve"):
            csl = slice(c * TCH, (c + 1) * TCH)
            for g in heads:
                for par, mw in ((0, 97), (1, 96)):
                    pn = ps_m.tile([P, TCH], F32, tag="ps_m", name="ps_m")
                    nc.tensor.matmul(pn[:mw, :], lhsT=gpk[:, g, par, :, :mw],
                                     rhs=zpk[:, :, csl], start=True, stop=True,
                                     perf_mode=PM.DoubleRow)
                    if drain == "dve":
                        nc.vector.tensor_copy(out=nhpk[0:mw, par, g, :],
                                              in_=pn[:mw, :])
                    else:
                        nc.scalar.copy(out=nhpk[0:mw, par, g, :], in_=pn[:mw, :])

        def alloc_y1(c):
            for ts in range(TSUB):
                ti = c * TSUB + ts
                y1_tiles[ti] = y1p.tile([P, D], F32, name=f"y1_{ti}")

        def emit_ln2(c, mv2, rstd2):
            ln_rstd_batch(mv2, rstd2, TSUB)
            if c == NCH - 1:
                # last chunk: prewarm the gelu table while DVE runs ln_z (for
                # other chunks the load hides under the next exp stream)
                nc.scalar.activation(out=tw, in_=eps_sb, func=AF.Gelu)
            n2pk = n2_pool.tile([P, 2, TCH], FP8, tag="n2pk", name="n2pk")
            if c == 0:
                # pad rows (d >= 192): zero once; fp8 garbage here would
                # poison fc1 psum via NaN * 0
                nc.gpsimd.memset(n2pk[96:128, :, :], 0.0)
            for ts in range(TSUB):
                ti = c * TSUB + ts
                z2 = ln_z(y1_tiles[ti], mv2[:, ts, :], rstd2[:, ts:ts + 1],
                          f"z2_{ts}")
                transpose_pack(z2, n2pk, ts * P)
            n2pk_tiles[c] = n2pk

        # --- two-deep software pipeline: scores/exps of chunk c+1 are issued
        # during chunk c's ctx/LN2/MLP windows so ACT/DVE never idle there.
        emit_phase_a_batch(0, 2)
        emit_phase_a_batch(2, 2)
        emit_nh(0)
        alloc_y1(0)
        emit_phase_a_batch(4, 4)
        emit_scores_head(0, 0, ks=[0, 1, 2, 3])
        emit_phase_a_batch(8, 4)
        emit_scores_head(0, 0, ks=[4, 5])
        emit_scores_head(0, 1, ks=[0, 1, 2, 3])
        emit_phase_a_batch(12, 4)
        emit_scores_head(0, 0, ks=[6, 7])
        emit_scores_head(0, 1, ks=[4, 5, 6, 7])
        emit_ctx_head(0, 0)
        emit_scores_head(0, 2)
        emit_nh(1)
        emit_ctx_head(0, 1)
        for c in range(NCH):
            mv2 = work.tile([P, TSUB, 2], F32, tag="mv2")
            rstd2 = work.tile([P, TSUB], F32, tag="rstd2")
            if c + 1 < NCH:
                alloc_y1(c + 1)
                emit_scores_head(c + 1, 0)
            emit_ctx_head(c, 2, mv2=mv2)
            emit_ln2(c, mv2, rstd2)
            if c + 1 < NCH:
                emit_scores_head(c + 1, 1)
            ht = emit_fc1(c)
            emit_fc2(c, ht)
            if c + 1 < NCH:
                emit_ctx_head(c + 1, 0)
                emit_scores_head(c + 1, 2)
                if c + 2 < NCH:
                    emit_nh(c + 2)
                emit_ctx_head(c + 1, 1)

    nc.finalize()
    return nc


_module_cache = {}


def kernel(**inputs):
    global LAST_RESULTS
    x = np.ascontiguousarray(np.asarray(inputs["x"], np.float32))
    B = x.shape[0]
    assert x.shape == (B, T, D) and B == 8

    weights, has_b2 = _prep_host(inputs)

    _install_table_patch()
    if has_b2 not in _module_cache:
        _module_cache[has_b2] = _build(has_b2)
    nc = _module_cache[has_b2]

    in_maps = [dict(weights, x=x[b]) for b in range(B)]
    res = run_bass_kernel_spmd(nc, in_maps, core_ids=list(range(B)), trace=TRACE)
    LAST_RESULTS = res
    out = np.stack([np.asarray(res.results[b]["y"], np.float32) for b in range(B)])
    return out
